# revision 12
# baseline (speedup 1.0000x reference)
"""Trainium2 Bass kernel for DigitConvolutionalModel.

Reference computation (B = 32768):
    x: [B, 784] -> reshape [B, 28, 28]
    conv 3x3 valid with w_conv -> [B, 26, 26] -> [B, 676]
    h1 = relu(conv @ W1 + b1)    W1: [676, 100]
    h2 = relu(h1 @ W2 + b2)      W2: [100, 100]
    out = h2 @ W3 + b3           W3: [100, 10]

Strategy
--------
Pure data parallel: batch split 8 ways (4096 rows/core), weights replicated.
The conv is linear, so it is folded into W1 on the host:
    conv(x) @ W1 == x @ (M @ W1) = x @ W1e,  W1e: [784, 100]
removing the conv from the device entirely (exact up to fp rounding).

On-device layout is "transposed": features on SBUF partitions, batch on the
free dimension, so each layer's PSUM output feeds the next matmul directly
as the moving operand. The host pre-transposes x per core and blocks it as
[n_dma_tiles, 128, KC, NTD] (contraction split 784 = 6*128 + 16; the 16-row
tail is loaded once as a [16, B_LOC] resident tile) so every x DMA uses all
128 partitions with long contiguous runs.

Matmuls run as float32r (TF32-like single-pass mode: ~280 ns per N=512
matmul warm). The kernel is HBM-bandwidth bound streaming x (~12.8 MB/core).
"""

import numpy as np

N_CORES = 8
B = 32768
B_LOC = B // N_CORES          # 4096 rows per core
NT = 512                      # matmul moving-dim tile (fp32 max free dim)
NTD = 1024                    # batch columns per x DMA tile
N_DMA = B_LOC // NTD          # 4
HALves = NTD // NT            # 2
KC = 6                        # full 128-row contraction chunks
KT = 784 - KC * 128           # 16-row tail
H = 100                       # hidden width
O = 10                        # output width
XBUFS = 3                     # x tile double-buffer depth

_COMPILED = {}
LAST_RESULTS = None


def _build_nc():
    import concourse.mybir as mybir
    from concourse import bacc
    from concourse.tile import TileContext

    f32 = mybir.dt.float32
    fr = mybir.dt.float32r

    nc = bacc.Bacc(
        "TRN2", target_bir_lowering=False, debug=False, num_devices=N_CORES
    )
    xt = nc.dram_tensor("xt", [N_DMA, 128, KC, NTD], fr, kind="ExternalInput")
    xtl = nc.dram_tensor("xtl", [KT, B_LOC], fr, kind="ExternalInput")
    w1 = nc.dram_tensor("w1", [128, KC, H], fr, kind="ExternalInput")
    w1l = nc.dram_tensor("w1l", [KT, H], fr, kind="ExternalInput")
    # packed [100, 110]: W2 | W3
    w23 = nc.dram_tensor("w23", [H, H + O], fr, kind="ExternalInput")
    # packed [100, 3]: b1 | b2 | b3 (b3 on partitions 0..9)
    bb = nc.dram_tensor("bb", [H, 3], f32, kind="ExternalInput")
    ot = nc.dram_tensor("ot", [O, B_LOC], f32, kind="ExternalOutput")

    relu = mybir.ActivationFunctionType.Relu
    add = mybir.AluOpType.add
    amax = mybir.AluOpType.max

    with TileContext(nc) as tc:
        with (
            tc.tile_pool(name="wpool", bufs=1) as wpool,
            tc.tile_pool(name="xpool", bufs=XBUFS) as xpool,
            tc.tile_pool(name="hpool", bufs=3) as hpool,
            tc.tile_pool(name="opool", bufs=3) as opool,
            tc.tile_pool(name="ppool", bufs=2, space="PSUM") as ppool,
        ):
            # x streams on the sync HWDGE ring in per-chunk DMAs (512 KB
            # each) so the first matmul only waits for the first chunk;
            # weights go on the scalar HWDGE ring. W1 is first (the first
            # matmul needs it); the rest is emitted after tile 0's x DMAs
            # to keep startup bandwidth contention low.
            w1_t = wpool.tile([128, KC, H], fr)
            nc.scalar.dma_start(out=w1_t, in_=w1.ap())

            w1l_t = wpool.tile([KT, H], fr)
            xl_t = wpool.tile([KT, B_LOC], fr)
            w23_t = wpool.tile([H, H + O], fr)
            bb_t = wpool.tile([H, 3], f32)

            w2_t = w23_t[:, 0:H]
            w3_t = w23_t[:, H : H + O]
            b1_t = bb_t[:, 0:1]
            b2_t = bb_t[:, 1:2]
            b3_t = bb_t[:O, 2:3]

            for t in range(N_DMA):
                x_t = xpool.tile([128, KC, NTD], fr, tag="xt")
                for c in range(KC):
                    nc.sync.dma_start(
                        out=x_t[:, c, :], in_=xt.ap()[t, :, c, :]
                    )
                if t == 0:
                    nc.scalar.dma_start(out=w1l_t, in_=w1l.ap())
                    nc.scalar.dma_start(out=xl_t, in_=xtl.ap())
                    nc.scalar.dma_start(out=w23_t, in_=w23.ap())
                    nc.scalar.dma_start(out=bb_t, in_=bb.ap())
                ps1h = []
                for hf in range(HALves):
                    ps1 = ppool.tile([128, NT], f32, tag="ps1", bufs=3)
                    ps1h.append(ps1)
                # interleave halves per chunk: each arriving chunk feeds
                # both halves' accumulating matmuls immediately
                for c in range(KC):
                    for hf in range(HALves):
                        n0 = hf * NT
                        nc.tensor.matmul(
                            ps1h[hf][:H, :],
                            lhsT=w1_t[:, c, :],
                            rhs=x_t[:, c, n0 : n0 + NT],
                            start=(c == 0),
                            stop=False,
                        )
                for hf in range(HALves):
                    n0 = hf * NT
                    gb = t * NTD + n0  # global batch offset within core
                    ps1 = ps1h[hf]
                    nc.tensor.matmul(
                        ps1[:H, :],
                        lhsT=w1l_t,
                        rhs=xl_t[:, gb : gb + NT],
                        start=False,
                        stop=True,
                    )
                    h1 = hpool.tile([H, NT], fr, tag="h1")
                    nc.vector.tensor_scalar(h1, ps1[:H, :], b1_t, 0.0, add, amax)

                    ps2 = ppool.tile([128, NT], f32, tag="ps2", bufs=2)
                    nc.tensor.matmul(
                        ps2[:H, :], lhsT=w2_t, rhs=h1, start=True, stop=True
                    )
                    h2 = hpool.tile([H, NT], fr, tag="h2")
                    nc.vector.tensor_scalar(h2, ps2[:H, :], b2_t, 0.0, add, amax)

                    ps3 = ppool.tile([128, NT], f32, tag="ps3", bufs=2)
                    nc.tensor.matmul(
                        ps3[:O, :], lhsT=w3_t, rhs=h2, start=True, stop=True
                    )
                    o_t = opool.tile([O, NT], f32)
                    nc.vector.tensor_scalar_add(o_t, ps3[:O, :], b3_t)
                    nc.scalar.dma_start(
                        out=ot.ap()[:, gb : gb + NT], in_=o_t
                    )

    nc.finalize()
    return nc


def _fold_conv_into_w1(w_conv, W1):
    """W1e[784, 100] such that x @ W1e == conv3x3(x) @ W1 (exact linear fold)."""
    W1e = np.zeros((28, 28, H), np.float64)
    W1r = W1.astype(np.float64).reshape(26, 26, H)
    wc = w_conv.astype(np.float64)
    for di in range(3):
        for dj in range(3):
            W1e[di : di + 26, dj : dj + 26, :] += wc[di, dj] * W1r
    return W1e.reshape(784, H).astype(np.float32)


def kernel(x, w_conv, W1, b1, W2, b2, W3, b3):
    from concourse.bass_utils import run_bass_kernel_spmd

    global LAST_RESULTS

    x = np.ascontiguousarray(np.asarray(x, np.float32))
    W1e = _fold_conv_into_w1(np.asarray(w_conv), np.asarray(W1))
    # [784, 100]: rows 0..767 -> [128, KC, 100]; rows 768..783 -> [16, 100]
    w1_dev = np.ascontiguousarray(
        W1e[: KC * 128].reshape(KC, 128, H).transpose(1, 0, 2)
    )
    w1l_dev = np.ascontiguousarray(W1e[KC * 128 :])
    w23_dev = np.zeros((H, H + O), np.float32)
    w23_dev[:, 0:H] = np.asarray(W2, np.float32)
    w23_dev[:, H : H + O] = np.asarray(W3, np.float32)
    bb_dev = np.zeros((H, 3), np.float32)
    bb_dev[:, 0] = np.asarray(b1, np.float32)
    bb_dev[:, 1] = np.asarray(b2, np.float32)
    bb_dev[:O, 2] = np.asarray(b3, np.float32)

    in_maps = []
    for c in range(N_CORES):
        xs = x[c * B_LOC : (c + 1) * B_LOC]          # [B_LOC, 784]
        xT = xs.T                                     # [784, B_LOC] (view)
        # main: [N_DMA, 128, KC, NTD], element [t, p, k, n]
        #       = xT[k*128 + p, t*NTD + n]
        xmain = np.ascontiguousarray(
            xT[: KC * 128]
            .reshape(KC, 128, N_DMA, NTD)
            .transpose(2, 1, 0, 3)
        )
        xtail = np.ascontiguousarray(xT[KC * 128 :])  # [16, B_LOC]
        in_maps.append(
            {
                "xt": xmain,
                "xtl": xtail,
                "w1": w1_dev,
                "w1l": w1l_dev,
                "w23": w23_dev,
                "bb": bb_dev,
            }
        )

    if "nc" not in _COMPILED:
        _COMPILED["nc"] = _build_nc()
    nc = _COMPILED["nc"]

    res = run_bass_kernel_spmd(nc, in_maps, core_ids=list(range(N_CORES)))
    LAST_RESULTS = res

    out = np.empty((B, O), np.float32)
    for c in range(N_CORES):
        out[c * B_LOC : (c + 1) * B_LOC] = res.results[c]["ot"].T
    return out
